# revision 37
# baseline (speedup 1.0000x reference)
"""Trainium2 Bass kernel for vectorized Cubify (nn_Cubify_18880676233661).

Contract: kernel(voxel_probas=[4,96,96,96] f32) -> (verts, faces, face_mask)
  verts:     [(97)^3, 3] f32   batch-invariant constant (host numpy)
  faces:     [6*96^3*2, 3] i32 batch-invariant constant (host numpy)
  face_mask: [4, 6*96^3*2] f32 computed on 8 NeuronCores

Sharding: (batch, h-half) -> 8 cores, all running one SPMD program. Each core
receives p3[z, k, h, w] = three z-shift components (aligned, z-1, z+1) of its
h-slab, with +-1 halo rows in h and halo columns in w (all wraparounds applied
host-side), and computes all 6 face directions:
active = occ & ~neighbor, duplicated x2 along the triangle axis, producing
fm [6, 96, 48, 96, 2] f32. z lives on the SBUF partition axis; z+-1 neighbors
come from the extra z-shifted input components (engine SBUF access patterns
cannot start at unaligned partitions, so on-chip partition shifts are
impossible); h+-1/w+-1 are free-dim shifts into the halos.

Per core (cost model): ~86.5us, vs the ~75us HBM roofline for its 26.9MB of
DMA traffic (in 5.6MB + out 21.2MB at ~358GB/s/core). VectorE does the
compares (1x f32), ScalarE the x2 duplication copies, and the output DMAs are
split across the SP HWDGE and Pool SWDGE rings.
"""

import numpy as np

_D = 96
_HH = 48          # h-rows per core
_SLAB = _HH + 2   # with h halo
_N = 4
_NCORES = 8

_nc_cache = None
_consts_cache = None


def _build_nc():
    import concourse.bass as bass
    import concourse.mybir as mybir
    from concourse.tile import TileContext

    f32 = mybir.dt.float32
    bf16 = mybir.dt.bfloat16
    gt = mybir.AluOpType.is_gt
    D, HH, W = _D, _HH, _D

    WP = W + 2  # w with halo columns (host-provided wrap)
    nc = bass.Bass()
    # p3[z, k, h, w] host-prepared: k = z-shift component (z-1, z, z+1), h has
    # a +-1 halo (50 rows), w has a +-1 halo (98 cols); all wraps pre-applied.
    p3 = nc.declare_dram_parameter("p3", [D, 3, _SLAB, WP], f32, isOutput=False)
    fm = nc.declare_dram_parameter("fm", [6, D, HH, W, 2], f32, isOutput=True)

    def bc(ap):
        # (..., w) -> (..., w, 2) stride-0 duplicate for the 2-triangle axis
        return ap.unsqueeze(-1).broadcast_to(tuple(ap.shape) + (2,))

    with TileContext(nc) as tc:
        with (
            tc.tile_pool(name="io", bufs=1) as io_pool,
            tc.tile_pool(name="act", bufs=2) as act_pool,
            tc.tile_pool(name="out", bufs=3) as out_pool,
        ):
            # component order in p3: [aligned, z-1, z+1] so the two shifted
            # components are contiguous -> one DMA (8 DMAs total = 8 HW sem
            # lanes, no lane reuse waits).
            pt = io_pool.tile([D, 3, _SLAB, WP], f32)  # [z, k, h, w]
            # Load + threshold the aligned component first so 4 of the 6
            # directions can start while the z-shifted components stream in.
            nc.sync.dma_start(out=pt[:, 0], in_=p3[:, 0])
            nc.vector.tensor_scalar(pt[:, 0], pt[:, 0], 0.5, None, gt)
            nc.sync.dma_start(out=pt[:, 1:3], in_=p3[:, 1:3])
            nc.vector.tensor_scalar(pt[:, 1:3], pt[:, 1:3], 0.5, None, gt)
            occ = pt[:, 0]             # [96, 50, 98] 0/1 occupancy with halos
            main = occ[:, 1:49, 1:97]  # this core's 48x96 h/w window
            nbs = {
                0: pt[:, 1, 1:49, 1:97],  # z-1 (shifted component)
                1: pt[:, 2, 1:49, 1:97],  # z+1
                2: occ[:, 0:48, 1:97],    # h-1 (halo row)
                3: occ[:, 2:50, 1:97],    # h+1
                4: occ[:, 1:49, 0:96],    # w-1 (halo col)
                5: occ[:, 1:49, 2:98],    # w+1
            }
            # First direction: two h-half fused-dup TTs so the first output
            # DMA launches while the z-shift components still stream in -
            # keeps the HBM pipe busy end to end (the kernel is DMA-roofline
            # bound in steady state).
            od2 = out_pool.tile([D, HH, W, 2], f32, tag="od")
            nc.vector.tensor_tensor(
                od2[:, 0:24], bc(main[:, 0:24]), bc(nbs[2][:, 0:24]), gt)
            nc.gpsimd.dma_start(out=fm[2, :, 0:24], in_=od2[:, 0:24])
            nc.vector.tensor_tensor(
                od2[:, 24:48], bc(main[:, 24:48]), bc(nbs[2][:, 24:48]), gt)
            nc.gpsimd.dma_start(out=fm[2, :, 24:48], in_=od2[:, 24:48])
            nc.vector.memset(od2[0:1, 0:1, 0:1, :], 0.0)
            nc.vector.memset(od2[0:1, 24:25, 0:1, :], 0.0)

            for i, d in enumerate((3, 4, 5, 0, 1)):
                act = act_pool.tile([D, HH, W], bf16, tag="act")
                od = out_pool.tile([D, HH, W, 2], f32, tag="od")
                # DVE: compare (1x, FD 4608); ACT: duplicate x2 into [w, 2];
                # output DMAs alternate between the SP HWDGE ring (which also
                # carries the inputs) and the Pool SWDGE ring.
                nc.vector.tensor_tensor(act[:], main, nbs[d], gt)
                nc.scalar.copy(od[:], bc(act[:]))
                dma_eng = nc.sync if i % 2 == 0 else nc.gpsimd
                dma_eng.dma_start(out=fm[d], in_=od[:])
                # Tiny WAR consumer: lets DVE observe this DMA's semaphore so
                # the kernel-tail drain funnels through a single sem wait.
                nc.vector.memset(od[0:1, 0:1, 0:1, :], 0.0)
    _strip_redundant_waits(nc)
    return nc


def _strip_redundant_waits(nc):
    """Reduce per-instruction sync waits to fit walrus' 1-wait ISA slots.

    Provably-safe reductions (guarantees computed on ORIGINAL wait sets):
      1. Same-engine order: a wait on this engine's own completion-count sem
         with value <= prior same-engine updates is guaranteed by in-order
         execution (+ the per-op pipeline drain).
      2. Transitive implication: (S >= v) is implied by another kept wait
         whose guarantee-closure contains it. The closure follows
         engine-completion sems (E >= k covers everything the first k
         instructions of E waited on) and DMA-completion sems (S >= 16n
         covers everything the first n DMAs updating S waited on, since a
         DMA's sem fires only after the DMA ran, which required its waits).
    """
    insts = []
    for blk in nc.m.functions[0].blocks:
        insts.extend(blk.instructions)
    updaters = {}   # sem id -> set of engines
    streams = {}    # engine -> list of inst
    for inst in insts:
        eng = str(inst.engine)
        streams.setdefault(eng, []).append(inst)
        si = inst.sync_info
        if si is None:
            continue
        for u in si.on_update:
            updaters.setdefault(u.id, set()).add(eng)

    orig_waits = {id(i): [(w.id, w.wait_value) for w in i.sync_info.on_wait]
                  for i in insts if i.sync_info is not None}

    # Engine-order completion sems: exclusively updated by this engine,
    # always +1, by >= 3 instructions.
    own_sem = {}
    for eng, stream in streams.items():
        counts, ok_vals = {}, {}
        for i in stream:
            if not i.sync_info:
                continue
            for u in i.sync_info.on_update:
                counts[u.id] = counts.get(u.id, 0) + 1
                ok_vals[u.id] = ok_vals.get(u.id, True) and u.update_value == 1
        best = None
        for sid, n in counts.items():
            if (n >= 3 and ok_vals[sid] and updaters.get(sid) == {eng}
                    and (best is None or n > counts[best])):
                best = sid
        if best is not None:
            own_sem[eng] = best
    sem_to_eng = {sid: eng for eng, sid in own_sem.items()}

    # observed_orig[eng][k] = original-waits union of E's first k own-sem
    # updating instructions (direct, not closed).
    observed_orig = {}
    for eng, stream in streams.items():
        sid_own = own_sem.get(eng)
        if sid_own is None:
            continue
        acc, hist = {}, [dict()]
        for inst in stream:
            si = inst.sync_info
            if si is None:
                continue
            for (sid, v) in orig_waits.get(id(inst), ()):
                if acc.get(sid, -1) < v:
                    acc = dict(acc)
                    acc[sid] = v
            if any(u.id == sid_own for u in si.on_update):
                hist.append(acc)
        observed_orig[eng] = hist

    # dma_sems[sid] = ordered list of (cum_value_after, original waits) for
    # the instructions updating sid asynchronously (not engine-order sems).
    # HWDGE/SWDGE execute FIFO per issuing engine; updates to one sem lane
    # here all come from the same queue in program order.
    dma_sems = {}
    for eng, stream in streams.items():
        for inst in stream:
            si = inst.sync_info
            if si is None:
                continue
            for u in si.on_update:
                if u.id in sem_to_eng or u.update_value == 1:
                    continue
                lst = dma_sems.setdefault(u.id, [])
                cum = (lst[-1][0] if lst else 0) + u.update_value
                lst.append((cum, orig_waits.get(id(inst), [])))

    def closure(waits):
        """All (sid -> min guaranteed value) implied by `waits` holding."""
        result = {}
        frontier = list(waits)
        while frontier:
            sid, v = frontier.pop()
            if result.get(sid, -1) >= v:
                continue
            result[sid] = v
            eng = sem_to_eng.get(sid)
            if eng is not None and eng in observed_orig:
                hist = observed_orig[eng]
                k = min(v, len(hist) - 1)
                frontier.extend(hist[k].items())
            elif sid in dma_sems:
                for cum, ws in dma_sems[sid]:
                    if cum <= v:
                        frontier.extend(ws)
        return result

    # Pass 1: same-engine order waits.
    for eng, stream in streams.items():
        cum = {}
        for inst in stream:
            si = inst.sync_info
            if si is None:
                continue
            if len(si.on_wait) > 1:
                keep = [w for w in si.on_wait
                        if not (w.id == own_sem.get(eng)
                                and cum.get(w.id, 0) >= w.wait_value)]
                if len(keep) != len(si.on_wait):
                    si.on_wait = keep
            for u in si.on_update:
                cum[u.id] = cum.get(u.id, 0) + u.update_value

    # Pass 2: drop waits implied by the closure of the other waits.
    for inst in insts:
        si = inst.sync_info
        if si is None or len(si.on_wait) <= 1:
            continue
        waits = list(si.on_wait)
        changed = True
        while changed and len(waits) > 1:
            changed = False
            for i, w in enumerate(waits):
                others = [(x.id, x.wait_value) for j, x in enumerate(waits)
                          if j != i]
                if closure(others).get(w.id, -1) >= w.wait_value:
                    waits.pop(i)
                    changed = True
                    break
        if len(waits) != len(si.on_wait):
            si.on_wait = waits


def _get_nc():
    global _nc_cache
    if _nc_cache is None:
        _nc_cache = _build_nc()
    return _nc_cache


def _constants():
    """Batch-invariant verts/faces tables, mirroring the reference."""
    global _consts_cache
    if _consts_cache is not None:
        return _consts_cache
    D = H = W = _D
    quad = np.array([
        [[0, 0, 0], [0, 0, 1], [0, 1, 0], [0, 1, 1]],
        [[1, 0, 0], [1, 0, 1], [1, 1, 0], [1, 1, 1]],
        [[1, 0, 0], [1, 0, 1], [0, 0, 0], [0, 0, 1]],
        [[0, 1, 0], [0, 1, 1], [1, 1, 0], [1, 1, 1]],
        [[1, 0, 0], [0, 0, 0], [1, 1, 0], [0, 1, 0]],
        [[0, 0, 1], [1, 0, 1], [0, 1, 1], [1, 1, 1]],
    ], dtype=np.int64)
    tri = quad[:, [[0, 1, 2], [1, 2, 3]], :]  # [6, 2, 3, 3]

    zz, yy, xx = np.meshgrid(np.arange(D), np.arange(H), np.arange(W), indexing="ij")
    cz = zz[None, :, :, :, None, None] + tri[:, None, None, None, :, :, 0]
    cy = yy[None, :, :, :, None, None] + tri[:, None, None, None, :, :, 1]
    cx = xx[None, :, :, :, None, None] + tri[:, None, None, None, :, :, 2]
    cid = (cz * (H + 1) + cy) * (W + 1) + cx
    faces = cid.reshape(-1, 3).astype(np.int32)

    gz, gy, gx = np.meshgrid(np.arange(D + 1), np.arange(H + 1), np.arange(W + 1),
                             indexing="ij")
    verts = (np.stack([gz, gy, gx], axis=-1).reshape(-1, 3).astype(np.float32) - 0.5)
    _consts_cache = (verts, faces)
    return _consts_cache


def _make_in_maps(vox):
    in_maps = []
    for c in range(_NCORES):
        b, hh = divmod(c, 2)
        h0 = hh * _HH
        idx = np.arange(h0 - 1, h0 + _HH + 1) % _D
        slab = vox[b][:, idx, :]                      # [96, 50, 96]
        wpad = np.concatenate([slab[:, :, -1:], slab, slab[:, :, :1]], axis=2)
        zpad = np.concatenate([wpad[-1:], wpad, wpad[:1]], axis=0)  # [98,50,98]
        # [z, k, h, w] with k = (aligned, z-1, z+1)
        p3 = np.stack([zpad[1:_D + 1], zpad[0:_D], zpad[2:_D + 2]], axis=1)
        in_maps.append({"p3": np.ascontiguousarray(p3)})
    return in_maps


_cache_installed = False


def _install_neff_cache():
    """Content-hash NEFF cache so repeat processes skip the walrus compile."""
    global _cache_installed
    if _cache_installed:
        return
    _cache_installed = True
    import hashlib
    import os
    import shutil
    import concourse.bass_utils as bu
    import concourse.bass2jax as b2j

    orig = bu.compile_bir_kernel

    def _normalized(bir_json):
        # Debug info embeds absolute source paths/line numbers; strip it so
        # the same program hashes identically regardless of where kernel.py
        # lives.
        import json

        def strip(o):
            if isinstance(o, dict):
                return {k: strip(v) for k, v in o.items() if k != "debug"}
            if isinstance(o, list):
                return [strip(v) for v in o]
            return o

        try:
            return json.dumps(strip(json.loads(bir_json)),
                              sort_keys=True).encode()
        except Exception:
            return bytes(bir_json)

    def cached(bir_json, tmpdir, neff_name="file.neff"):
        h = hashlib.sha256(_normalized(bir_json) + neff_name.encode()).hexdigest()
        cdir = "/var/tmp/bass_neff_cache"
        cpath = os.path.join(cdir, h + ".neff")
        if os.path.exists(cpath):
            dst = os.path.join(tmpdir, neff_name)
            shutil.copy(cpath, dst)
            return dst
        out = orig(bir_json, tmpdir, neff_name=neff_name)
        try:
            os.makedirs(cdir, exist_ok=True)
            shutil.copy(out, cpath + ".tmp")
            os.replace(cpath + ".tmp", cpath)
        except OSError:
            pass
        return out

    bu.compile_bir_kernel = cached
    b2j.compile_bir_kernel = cached


def _run(vox, trace=False, trace_kwargs=None):
    from concourse.bass_utils import run_bass_kernel_spmd

    _install_neff_cache()
    nc = _get_nc()
    in_maps = _make_in_maps(vox)
    kw = {}
    if trace:
        kw = dict(trace=True, **(trace_kwargs or {}))
    out = run_bass_kernel_spmd(nc, in_maps, list(range(_NCORES)), **kw)
    fm = np.empty((_N, 6, _D, _D, _D, 2), np.float32)
    for c in range(_NCORES):
        b, hh = divmod(c, 2)
        fm[b, :, :, hh * _HH:(hh + 1) * _HH] = out.results[c]["fm"]
    return fm.reshape(_N, -1), out


def kernel(voxel_probas):
    vox = np.asarray(voxel_probas, dtype=np.float32)
    assert vox.shape == (_N, _D, _D, _D), vox.shape
    face_mask, _ = _run(vox)
    verts, faces = _constants()
    return verts, faces, face_mask
